# revision 2
# baseline (speedup 1.0000x reference)
"""Trainium2 Bass kernel for nn_My_maxpool1 (gnn_message_passing).

Contract: kernel(**inputs) takes FULL inputs (traindata [1.5M,4] f32,
neighbor [2M,5] f32) and returns the FULL output [262144, 4] f32,
running the gather/reduce/recurrence on 8 NeuronCores.

Host side (sharding prep, per the problem's sharding hint):
  - global stable descending sort of neighbor by col 1, keep bottom-n rows
    (this reproduces tf.nn.top_k's ordering exactly);
  - shard the n selected rows across 8 cores by contiguous group blocks;
  - each core receives the full traindata table (gather source) plus one
    packed int32 tensor [128, 2K+4]: its shard's ids (col 0 of the selected
    rows), the raw bits of their flags (col 4), and the raw bits of
    traindata[0] (replicated per partition; used for the never-updated
    sentinel groups).

Device side (per core, SPMD):
  - indirect-DMA gather rows = traindata[ids]           [128, K, 4]
  - rmax = max(rows[:, :, 1:4], axis=-1)                [128, K]
  - 4-step group recurrence (groups = consecutive fours along k):
      re = (flag_j != 0) == (rmax_j > maxmin)
      maxmin  = where(re, rmax_j, maxmin)
      maxidxf = where(re, id_j, maxidxf)
  - out rows: init traindata[0] (the clip(-100,0) sentinel case), then
    for j: where(maxidxf == id_j, rows_j) — selects traindata[maxindex]
    without a second gather.

Note: the reference's rev/td/searchsorted machinery is an exact identity
here (traindata col 0 == arange(T), all lookup keys < T, first match is
at index == key, and appended rev rows are never read back), so tp == id.
"""

import numpy as np

import concourse.bass as bass
import concourse.mybir as mybir
from concourse.bass_utils import run_bass_kernel_spmd
from concourse.tile import TileContext

F32 = mybir.dt.float32
I32 = mybir.dt.int32
U32 = mybir.dt.uint32
AX = mybir.AxisListType
OP = mybir.AluOpType

N_CORES = 8
T = 1_500_000       # traindata rows
N = 2_000_000       # neighbor rows
n = 1_048_576       # selected rows (bottom-n of the descending sort)
P = 128             # SBUF partitions
E = n // N_CORES    # selected rows per core            = 131072
K = E // P          # selected rows per partition       = 1024
NCHUNKS = 1         # single completion sem for the scalar-indirect gathers

_cache: dict = {}


def _build_nc(T=T, K=K, nchunks=NCHUNKS):
    """Raw-bass build: explicit semaphores, no TileContext.

    Tile's auto-generated kernel-tail drain accumulates one sem wait per
    DMA lane used and walrus caps sync waits per instruction at ~3, so any
    multi-DMA version of this kernel fails codegen under Tile. Raw bass
    keeps every instruction at <=1 wait.
    """
    Q = K // 4
    G_CORE = P * Q
    assert K % (4 * nchunks) == 0
    W = K // nchunks
    nc = bass.Bass()
    td = nc.declare_dram_parameter("traindata", [T, 4], F32, isOutput=False)
    # packed: [0:K) ids i32 | [K:2K) flag bits | [2K:2K+4) traindata[0] bits
    pk_d = nc.declare_dram_parameter("packed", [P, 2 * K + 4], I32,
                                     isOutput=False)
    out_d = nc.declare_dram_parameter("out", [G_CORE, 4], F32, isOutput=True)

    from contextlib import ExitStack
    with ExitStack() as ctx:
        pk = ctx.enter_context(nc.sbuf_tensor([P, 2 * K + 4], I32))
        rows_t = ctx.enter_context(nc.sbuf_tensor([P, K * 4], F32))
        rmax = ctx.enter_context(nc.sbuf_tensor([P, K], F32))
        fl = ctx.enter_context(nc.sbuf_tensor([P, K], F32))
        maxmin = ctx.enter_context(nc.sbuf_tensor([P, Q], F32))
        maxmin2 = ctx.enter_context(nc.sbuf_tensor([P, Q], F32))
        maxidxf = ctx.enter_context(nc.sbuf_tensor([P, Q], F32))
        maxidxf2 = ctx.enter_context(nc.sbuf_tensor([P, Q], F32))
        gt = ctx.enter_context(nc.sbuf_tensor([P, Q], F32))
        re = ctx.enter_context(nc.sbuf_tensor([P, Q], U32))
        mk = ctx.enter_context(nc.sbuf_tensor([P, Q], U32))
        outsb = ctx.enter_context(nc.sbuf_tensor([P, Q * 4], F32))
        in_sem = ctx.enter_context(nc.semaphore("in_sem"))
        g_sems = [ctx.enter_context(nc.semaphore(f"g_sem{i}"))
                  for i in range(4)]
        v_sem = ctx.enter_context(nc.semaphore("v_sem"))
        out_sem = ctx.enter_context(nc.semaphore("out_sem"))
        block = ctx.enter_context(nc.Block())
        ids_sb = pk[:, 0:K]
        fl_raw = pk[:, K:2 * K].bitcast(F32)
        row0 = pk[:, 2 * K:2 * K + 4].bitcast(F32)
        rows = rows_t[:].rearrange("p (k f) -> p k f", f=4)
        rows_g = rows_t[:].rearrange("p (q j f) -> p q j f", j=4, f=4)
        rmax_g = rmax[:].rearrange("p (q j) -> p q j", j=4)
        fl_g = fl[:].rearrange("p (q j) -> p q j", j=4)
        outsb_v = outsb[:].rearrange("p (q f) -> p q f", f=4)

        @block.sync
        def _(sync):
            sync.dma_start(out=pk[:], in_=pk_d[:]).then_inc(in_sem, 16)
            outd_v = out_d[:].rearrange("(p q) f -> p (q f)", p=P)
            QW = Q // 4
            for i in range(4):
                sync.wait_ge(v_sem, i + 1)
                sync.dma_start(
                    out=outd_v[:, QW * 4 * i:QW * 4 * (i + 1)],
                    in_=outsb[:, QW * 4 * i:QW * 4 * (i + 1)],
                ).then_inc(out_sem, 16)
            sync.wait_ge(out_sem, 64)

        @block.gpsimd
        def _(gpsimd):
            # Scalar-dynamic-offset indirect DMA: ONE offset per partition per
            # instruction, contiguous 16B row fill. This is the only indirect
            # form this runtime executes correctly (the vector-offset path
            # returns swizzled garbage on HW); K instructions gather K rows
            # per partition.
            gpsimd.wait_ge(in_sem, 16)
            for k in range(K):
                gpsimd.indirect_dma_start(
                    out=rows_t[:, 4 * k:4 * k + 4],
                    out_offset=None,
                    in_=td[:],
                    in_offset=bass.IndirectOffsetOnAxis(
                        ap=ids_sb[:, k:k + 1], axis=0),
                ).then_inc(g_sems[k // (K // 4)], 16)

        @block.vector
        def _(vector):
            vector.wait_ge(in_sem, 16)
            vector.tensor_scalar(
                out=fl[:], in0=fl_raw, scalar1=0.0, scalar2=None,
                op0=OP.not_equal,
            )
            vector.tensor_copy(
                out=outsb_v, in_=row0.unsqueeze(1).to_broadcast([P, Q, 4]),
            )
            vector.memset(maxmin[:], -100000.0)
            vector.memset(maxidxf[:], -100.0)
            vector.drain()
            KW = K // 4          # 256 slot-cols per quarter
            QW = Q // 4          # 64 group-cols per quarter
            for i in range(4):
                ks = slice(KW * i, KW * (i + 1))
                qs = slice(QW * i, QW * (i + 1))
                # all of this quarter's gathers inc g_sems[i] by 16 each;
                # wait the full quarter count (no partial waits: completion
                # order across instructions is not guaranteed)
                vector.wait_ge(g_sems[i], 16 * (K // 4))
                vector.tensor_reduce(
                    rmax[:, ks], rows[:, ks, 1:4], AX.X, OP.max
                )
                vector.drain()
                cur_mm, nxt_mm = maxmin, maxmin2
                cur_mi, nxt_mi = maxidxf, maxidxf2
                for j in range(4):
                    vector.tensor_tensor(
                        out=gt[:, qs], in0=rmax_g[:, qs, j],
                        in1=cur_mm[:, qs], op=OP.is_gt
                    )
                    vector.drain()
                    vector.tensor_tensor(
                        out=re[:, qs], in0=fl_g[:, qs, j], in1=gt[:, qs],
                        op=OP.is_equal
                    )
                    vector.drain()
                    vector.select(
                        out=nxt_mm[:, qs], mask=re[:, qs],
                        on_true=rmax_g[:, qs, j], on_false=cur_mm[:, qs],
                        add_drain=True,
                    )
                    vector.select(
                        out=nxt_mi[:, qs], mask=re[:, qs],
                        on_true=rows_g[:, qs, j, 0], on_false=cur_mi[:, qs],
                        add_drain=True,
                    )
                    vector.drain()
                    cur_mm, nxt_mm = nxt_mm, cur_mm
                    cur_mi, nxt_mi = nxt_mi, cur_mi
                for j in range(4):
                    vector.tensor_tensor(
                        out=mk[:, qs], in0=cur_mi[:, qs],
                        in1=rows_g[:, qs, j, 0], op=OP.is_equal
                    )
                    vector.drain()
                    for f in range(4):
                        vector.copy_predicated(
                            out=outsb_v[:, qs, f],
                            mask=mk[:, qs],
                            data=rows_g[:, qs, j, f],
                        )
                    vector.drain()
                vector.nop().then_inc(v_sem, 1)

    return nc


def _build_nc_tile(T=T, K=K, nchunks=NCHUNKS):
    Q = K // 4
    G_CORE = P * Q
    assert K % (4 * nchunks) == 0
    W = K // nchunks
    nc = bass.Bass()
    td = nc.declare_dram_parameter("traindata", [T, 4], F32, isOutput=False)
    # packed: [0:K) ids i32 | [K:2K) flag bits | [2K:2K+4) traindata[0] bits
    pk_d = nc.declare_dram_parameter("packed", [P, 2 * K + 4], I32,
                                     isOutput=False)
    out_d = nc.declare_dram_parameter("out", [G_CORE, 4], F32, isOutput=True)

    with TileContext(nc) as tc:
        with tc.tile_pool(name="main", bufs=1) as pool:
            pk = pool.tile([P, 2 * K + 4], I32)
            nc.sync.dma_start(out=pk[:], in_=pk_d[:])
            ids_sb = pk[:, 0:K]
            fl_raw = pk[:, K:2 * K].bitcast(F32)
            row0 = pk[:, 2 * K:2 * K + 4].bitcast(F32)

            # main gather, chunked so SWDGE descriptor gen overlaps transfers
            rows = pool.tile([P, K, 4], F32)
            for i in range(nchunks):
                rs = slice(i * W, (i + 1) * W)
                nc.gpsimd.indirect_dma_start(
                    out=rows[:, rs, :],
                    out_offset=None,
                    in_=td[:],
                    in_offset=bass.IndirectOffsetOnAxis(ap=ids_sb[:, rs], axis=0),
                )

            # rmax over feature cols 1:4, per chunk (overlaps later gathers)
            rmax = pool.tile([P, K], F32)
            for i in range(nchunks):
                rs = slice(i * W, (i + 1) * W)
                nc.vector.tensor_reduce(
                    rmax[:, rs], rows[:, rs, 1:4], AX.X, OP.max
                )

            fl = pool.tile([P, K], F32)
            nc.vector.tensor_scalar(
                out=fl[:], in0=fl_raw, scalar1=0.0, scalar2=None,
                op0=OP.not_equal,
            )

            # group views: [P, Q, 4] over k = 4q + j
            rmax_g = rmax[:].rearrange("p (q j) -> p q j", j=4)
            fl_g = fl[:].rearrange("p (q j) -> p q j", j=4)
            rows_g = rows[:].rearrange("p (q j) f -> p q j f", j=4)

            maxmin = pool.tile([P, Q], F32)
            nc.vector.memset(maxmin[:], -100000.0)
            maxidxf = pool.tile([P, Q], F32)
            nc.vector.memset(maxidxf[:], -100.0)
            gt = pool.tile([P, Q], F32)
            re = pool.tile([P, Q], U32)
            maxmin2 = pool.tile([P, Q], F32)
            maxidxf2 = pool.tile([P, Q], F32)

            cur_mm, nxt_mm = maxmin, maxmin2
            cur_mi, nxt_mi = maxidxf, maxidxf2
            for j in range(4):
                nc.vector.tensor_tensor(
                    out=gt[:], in0=rmax_g[:, :, j], in1=cur_mm[:], op=OP.is_gt
                )
                nc.vector.tensor_tensor(
                    out=re[:], in0=fl_g[:, :, j], in1=gt[:], op=OP.is_equal
                )
                nc.vector.select(
                    out=nxt_mm[:], mask=re[:],
                    on_true=rmax_g[:, :, j], on_false=cur_mm[:],
                )
                nc.vector.select(
                    out=nxt_mi[:], mask=re[:],
                    on_true=rows_g[:, :, j, 0], on_false=cur_mi[:],
                )
                cur_mm, nxt_mm = nxt_mm, cur_mm
                cur_mi, nxt_mi = nxt_mi, cur_mi

            # output rows
            outsb = pool.tile([P, Q * 4], F32)
            outsb_v = outsb[:].rearrange("p (q f) -> p q f", f=4)
            nc.vector.tensor_copy(
                out=outsb_v,
                in_=row0.unsqueeze(1).to_broadcast([P, Q, 4]),
            )
            mk = pool.tile([P, Q], U32)
            for j in range(4):
                nc.vector.tensor_tensor(
                    out=mk[:], in0=cur_mi[:], in1=rows_g[:, :, j, 0], op=OP.is_equal
                )
                for f in range(4):
                    nc.vector.copy_predicated(
                        out=outsb_v[:, :, f],
                        mask=mk[:],
                        data=rows_g[:, :, j, f],
                    )

            nc.sync.dma_start(
                out=out_d[:].rearrange("(p q) f -> p (q f)", p=P),
                in_=outsb[:],
            )

    return nc


def _get_nc():
    if "nc" not in _cache:
        _cache["nc"] = _build_nc()
    return _cache["nc"]


def _pack_core(ids_i32, flags_f32, row0_f32):
    """[P, K] int32 ids, [P, K] f32 flags, [4] f32 row0 -> [P, 2K+4] int32."""
    return np.concatenate(
        [ids_i32,
         flags_f32.view(np.int32),
         np.broadcast_to(row0_f32.view(np.int32), (P, 4))],
        axis=1,
    )


def kernel(traindata, neighbor, _trace=False):
    traindata = np.ascontiguousarray(np.asarray(traindata, dtype=np.float32))
    neighbor = np.asarray(neighbor, dtype=np.float32)
    assert traindata.shape == (T, 4) and neighbor.shape == (N, 5)

    # ---- host: global sort + shard (the sharding hint's "after the global
    # sort" prep) ----
    order = np.argsort(-neighbor[:, 1], kind="stable")
    sel = order[N - n:]
    ids = neighbor[sel, 0].astype(np.int32)
    flags = np.ascontiguousarray(neighbor[sel, 4])
    row0 = np.ascontiguousarray(traindata[0])

    nc = _get_nc()
    in_maps = []
    for c in range(N_CORES):
        s = slice(c * E, (c + 1) * E)
        in_maps.append({
            "traindata": traindata,
            "packed": np.ascontiguousarray(_pack_core(
                ids[s].reshape(P, K), flags[s].reshape(P, K), row0)),
        })
    res = run_bass_kernel_spmd(
        nc, in_maps, core_ids=list(range(N_CORES)), trace=_trace
    )
    _cache["last_results"] = res
    out = np.concatenate([r["out"] for r in res.results], axis=0)
    return np.ascontiguousarray(out.astype(np.float32))



# revision 3
# speedup vs baseline: 1.0194x; 1.0194x over previous
"""Trainium2 Bass kernel for nn_My_maxpool1 (gnn_message_passing).

Contract: kernel(**inputs) takes FULL inputs (traindata [1.5M,4] f32,
neighbor [2M,5] f32) and returns the FULL output [262144, 4] f32,
running the gather/reduce/recurrence on 8 NeuronCores.

Host side (sharding prep, per the problem's sharding hint):
  - global stable descending sort of neighbor by col 1, keep bottom-n rows
    (this reproduces tf.nn.top_k's ordering exactly);
  - shard the n selected rows across 8 cores by contiguous group blocks;
  - each core receives the full traindata table (gather source) plus one
    packed int32 tensor [128, 2K+4]: its shard's ids (col 0 of the selected
    rows), the raw bits of their flags (col 4), and the raw bits of
    traindata[0] (replicated per partition; used for the never-updated
    sentinel groups).

Device side (per core, SPMD):
  - indirect-DMA gather rows = traindata[ids]           [128, K, 4]
  - rmax = max(rows[:, :, 1:4], axis=-1)                [128, K]
  - 4-step group recurrence (groups = consecutive fours along k):
      re = (flag_j != 0) == (rmax_j > maxmin)
      maxmin  = where(re, rmax_j, maxmin)
      maxidxf = where(re, id_j, maxidxf)
  - out rows: init traindata[0] (the clip(-100,0) sentinel case), then
    for j: where(maxidxf == id_j, rows_j) — selects traindata[maxindex]
    without a second gather.

Note: the reference's rev/td/searchsorted machinery is an exact identity
here (traindata col 0 == arange(T), all lookup keys < T, first match is
at index == key, and appended rev rows are never read back), so tp == id.
"""

import numpy as np

import concourse.bass as bass
import concourse.mybir as mybir
from concourse.bass_utils import run_bass_kernel_spmd
from concourse.tile import TileContext

F32 = mybir.dt.float32
I32 = mybir.dt.int32
U32 = mybir.dt.uint32
AX = mybir.AxisListType
OP = mybir.AluOpType

N_CORES = 8
T = 1_500_000       # traindata rows
N = 2_000_000       # neighbor rows
n = 1_048_576       # selected rows (bottom-n of the descending sort)
P = 128             # SBUF partitions
E = n // N_CORES    # selected rows per core            = 131072
K = E // P          # selected rows per partition       = 1024
NCHUNKS = 1         # single completion sem for the scalar-indirect gathers

_cache: dict = {}


def _build_nc(T=T, K=K, nchunks=NCHUNKS):
    """Raw-bass build: explicit semaphores, no TileContext.

    Tile's auto-generated kernel-tail drain accumulates one sem wait per
    DMA lane used and walrus caps sync waits per instruction at ~3, so any
    multi-DMA version of this kernel fails codegen under Tile. Raw bass
    keeps every instruction at <=1 wait.
    """
    Q = K // 4
    G_CORE = P * Q
    assert K % (4 * nchunks) == 0
    W = K // nchunks
    nc = bass.Bass()
    td = nc.declare_dram_parameter("traindata", [T, 4], F32, isOutput=False)
    # packed: [0:K) ids i32 | [K:2K) flag bits | [2K:2K+4) traindata[0] bits
    pk_d = nc.declare_dram_parameter("packed", [P, 2 * K + 4], I32,
                                     isOutput=False)
    out_d = nc.declare_dram_parameter("out", [G_CORE, 4], F32, isOutput=True)

    from contextlib import ExitStack
    with ExitStack() as ctx:
        pk = ctx.enter_context(nc.sbuf_tensor([P, 2 * K + 4], I32))
        rows_t = ctx.enter_context(nc.sbuf_tensor([P, K * 4], F32))
        rmax = ctx.enter_context(nc.sbuf_tensor([P, K], F32))
        fl = ctx.enter_context(nc.sbuf_tensor([P, K], F32))
        maxmin = ctx.enter_context(nc.sbuf_tensor([P, Q], F32))
        maxmin2 = ctx.enter_context(nc.sbuf_tensor([P, Q], F32))
        maxidxf = ctx.enter_context(nc.sbuf_tensor([P, Q], F32))
        maxidxf2 = ctx.enter_context(nc.sbuf_tensor([P, Q], F32))
        gt = ctx.enter_context(nc.sbuf_tensor([P, Q], F32))
        re = ctx.enter_context(nc.sbuf_tensor([P, Q], U32))
        mk = ctx.enter_context(nc.sbuf_tensor([P, Q], U32))
        outsb = ctx.enter_context(nc.sbuf_tensor([P, Q * 4], F32))
        in_sem = ctx.enter_context(nc.semaphore("in_sem"))
        g_sems = [ctx.enter_context(nc.semaphore(f"g_sem{i}"))
                  for i in range(8)]
        v_sem = ctx.enter_context(nc.semaphore("v_sem"))
        out_sem = ctx.enter_context(nc.semaphore("out_sem"))
        block = ctx.enter_context(nc.Block())
        ids_sb = pk[:, 0:K]
        fl_raw = pk[:, K:2 * K].bitcast(F32)
        row0 = pk[:, 2 * K:2 * K + 4].bitcast(F32)
        rows = rows_t[:].rearrange("p (k f) -> p k f", f=4)
        rows_g = rows_t[:].rearrange("p (q j f) -> p q j f", j=4, f=4)
        rmax_g = rmax[:].rearrange("p (q j) -> p q j", j=4)
        fl_g = fl[:].rearrange("p (q j) -> p q j", j=4)
        outsb_v = outsb[:].rearrange("p (q f) -> p q f", f=4)

        @block.sync
        def _(sync):
            sync.dma_start(out=pk[:], in_=pk_d[:]).then_inc(in_sem, 16)
            outd_v = out_d[:].rearrange("(p q) f -> p (q f)", p=P)
            QW = Q // 8
            for i in range(8):
                sync.wait_ge(v_sem, i + 1)
                sync.dma_start(
                    out=outd_v[:, QW * 4 * i:QW * 4 * (i + 1)],
                    in_=outsb[:, QW * 4 * i:QW * 4 * (i + 1)],
                ).then_inc(out_sem, 16)
            sync.wait_ge(out_sem, 128)

        @block.gpsimd
        def _(gpsimd):
            # Scalar-dynamic-offset indirect DMA: ONE offset per partition per
            # instruction, contiguous 16B row fill. This is the only indirect
            # form this runtime executes correctly (the vector-offset path
            # returns swizzled garbage on HW); K instructions gather K rows
            # per partition.
            gpsimd.wait_ge(in_sem, 16)
            for k in range(K):
                gpsimd.indirect_dma_start(
                    out=rows_t[:, 4 * k:4 * k + 4],
                    out_offset=None,
                    in_=td[:],
                    in_offset=bass.IndirectOffsetOnAxis(
                        ap=ids_sb[:, k:k + 1], axis=0),
                ).then_inc(g_sems[k // (K // 8)], 16)

        @block.vector
        def _(vector):
            vector.wait_ge(in_sem, 16)
            vector.tensor_scalar(
                out=fl[:], in0=fl_raw, scalar1=0.0, scalar2=None,
                op0=OP.not_equal,
            )
            vector.tensor_copy(
                out=outsb_v, in_=row0.unsqueeze(1).to_broadcast([P, Q, 4]),
            )
            vector.memset(maxmin[:], -100000.0)
            vector.memset(maxidxf[:], -100.0)
            vector.drain()
            KW = K // 8          # 128 slot-cols per slice
            QW = Q // 8          # 32 group-cols per slice
            for i in range(8):
                ks = slice(KW * i, KW * (i + 1))
                qs = slice(QW * i, QW * (i + 1))
                # all of this quarter's gathers inc g_sems[i] by 16 each;
                # wait the full quarter count (no partial waits: completion
                # order across instructions is not guaranteed)
                vector.wait_ge(g_sems[i], 16 * (K // 8))
                vector.tensor_reduce(
                    rmax[:, ks], rows[:, ks, 1:4], AX.X, OP.max
                )
                vector.drain()
                cur_mm, nxt_mm = maxmin, maxmin2
                cur_mi, nxt_mi = maxidxf, maxidxf2
                for j in range(4):
                    vector.tensor_tensor(
                        out=gt[:, qs], in0=rmax_g[:, qs, j],
                        in1=cur_mm[:, qs], op=OP.is_gt
                    )
                    vector.drain()
                    vector.tensor_tensor(
                        out=re[:, qs], in0=fl_g[:, qs, j], in1=gt[:, qs],
                        op=OP.is_equal
                    )
                    vector.drain()
                    vector.select(
                        out=nxt_mm[:, qs], mask=re[:, qs],
                        on_true=rmax_g[:, qs, j], on_false=cur_mm[:, qs],
                        add_drain=True,
                    )
                    vector.select(
                        out=nxt_mi[:, qs], mask=re[:, qs],
                        on_true=rows_g[:, qs, j, 0], on_false=cur_mi[:, qs],
                        add_drain=True,
                    )
                    vector.drain()
                    cur_mm, nxt_mm = nxt_mm, cur_mm
                    cur_mi, nxt_mi = nxt_mi, cur_mi
                for j in range(4):
                    vector.tensor_tensor(
                        out=mk[:, qs], in0=cur_mi[:, qs],
                        in1=rows_g[:, qs, j, 0], op=OP.is_equal
                    )
                    vector.drain()
                    for f in range(4):
                        vector.copy_predicated(
                            out=outsb_v[:, qs, f],
                            mask=mk[:, qs],
                            data=rows_g[:, qs, j, f],
                        )
                    vector.drain()
                vector.nop().then_inc(v_sem, 1)

    return nc


def _build_nc_tile(T=T, K=K, nchunks=NCHUNKS):
    Q = K // 4
    G_CORE = P * Q
    assert K % (4 * nchunks) == 0
    W = K // nchunks
    nc = bass.Bass()
    td = nc.declare_dram_parameter("traindata", [T, 4], F32, isOutput=False)
    # packed: [0:K) ids i32 | [K:2K) flag bits | [2K:2K+4) traindata[0] bits
    pk_d = nc.declare_dram_parameter("packed", [P, 2 * K + 4], I32,
                                     isOutput=False)
    out_d = nc.declare_dram_parameter("out", [G_CORE, 4], F32, isOutput=True)

    with TileContext(nc) as tc:
        with tc.tile_pool(name="main", bufs=1) as pool:
            pk = pool.tile([P, 2 * K + 4], I32)
            nc.sync.dma_start(out=pk[:], in_=pk_d[:])
            ids_sb = pk[:, 0:K]
            fl_raw = pk[:, K:2 * K].bitcast(F32)
            row0 = pk[:, 2 * K:2 * K + 4].bitcast(F32)

            # main gather, chunked so SWDGE descriptor gen overlaps transfers
            rows = pool.tile([P, K, 4], F32)
            for i in range(nchunks):
                rs = slice(i * W, (i + 1) * W)
                nc.gpsimd.indirect_dma_start(
                    out=rows[:, rs, :],
                    out_offset=None,
                    in_=td[:],
                    in_offset=bass.IndirectOffsetOnAxis(ap=ids_sb[:, rs], axis=0),
                )

            # rmax over feature cols 1:4, per chunk (overlaps later gathers)
            rmax = pool.tile([P, K], F32)
            for i in range(nchunks):
                rs = slice(i * W, (i + 1) * W)
                nc.vector.tensor_reduce(
                    rmax[:, rs], rows[:, rs, 1:4], AX.X, OP.max
                )

            fl = pool.tile([P, K], F32)
            nc.vector.tensor_scalar(
                out=fl[:], in0=fl_raw, scalar1=0.0, scalar2=None,
                op0=OP.not_equal,
            )

            # group views: [P, Q, 4] over k = 4q + j
            rmax_g = rmax[:].rearrange("p (q j) -> p q j", j=4)
            fl_g = fl[:].rearrange("p (q j) -> p q j", j=4)
            rows_g = rows[:].rearrange("p (q j) f -> p q j f", j=4)

            maxmin = pool.tile([P, Q], F32)
            nc.vector.memset(maxmin[:], -100000.0)
            maxidxf = pool.tile([P, Q], F32)
            nc.vector.memset(maxidxf[:], -100.0)
            gt = pool.tile([P, Q], F32)
            re = pool.tile([P, Q], U32)
            maxmin2 = pool.tile([P, Q], F32)
            maxidxf2 = pool.tile([P, Q], F32)

            cur_mm, nxt_mm = maxmin, maxmin2
            cur_mi, nxt_mi = maxidxf, maxidxf2
            for j in range(4):
                nc.vector.tensor_tensor(
                    out=gt[:], in0=rmax_g[:, :, j], in1=cur_mm[:], op=OP.is_gt
                )
                nc.vector.tensor_tensor(
                    out=re[:], in0=fl_g[:, :, j], in1=gt[:], op=OP.is_equal
                )
                nc.vector.select(
                    out=nxt_mm[:], mask=re[:],
                    on_true=rmax_g[:, :, j], on_false=cur_mm[:],
                )
                nc.vector.select(
                    out=nxt_mi[:], mask=re[:],
                    on_true=rows_g[:, :, j, 0], on_false=cur_mi[:],
                )
                cur_mm, nxt_mm = nxt_mm, cur_mm
                cur_mi, nxt_mi = nxt_mi, cur_mi

            # output rows
            outsb = pool.tile([P, Q * 4], F32)
            outsb_v = outsb[:].rearrange("p (q f) -> p q f", f=4)
            nc.vector.tensor_copy(
                out=outsb_v,
                in_=row0.unsqueeze(1).to_broadcast([P, Q, 4]),
            )
            mk = pool.tile([P, Q], U32)
            for j in range(4):
                nc.vector.tensor_tensor(
                    out=mk[:], in0=cur_mi[:], in1=rows_g[:, :, j, 0], op=OP.is_equal
                )
                for f in range(4):
                    nc.vector.copy_predicated(
                        out=outsb_v[:, :, f],
                        mask=mk[:],
                        data=rows_g[:, :, j, f],
                    )

            nc.sync.dma_start(
                out=out_d[:].rearrange("(p q) f -> p (q f)", p=P),
                in_=outsb[:],
            )

    return nc


def _get_nc():
    if "nc" not in _cache:
        _cache["nc"] = _build_nc()
    return _cache["nc"]


def _pack_core(ids_i32, flags_f32, row0_f32):
    """[P, K] int32 ids, [P, K] f32 flags, [4] f32 row0 -> [P, 2K+4] int32."""
    return np.concatenate(
        [ids_i32,
         flags_f32.view(np.int32),
         np.broadcast_to(row0_f32.view(np.int32), (P, 4))],
        axis=1,
    )


def kernel(traindata, neighbor, _trace=False):
    traindata = np.ascontiguousarray(np.asarray(traindata, dtype=np.float32))
    neighbor = np.asarray(neighbor, dtype=np.float32)
    assert traindata.shape == (T, 4) and neighbor.shape == (N, 5)

    # ---- host: global sort + shard (the sharding hint's "after the global
    # sort" prep) ----
    order = np.argsort(-neighbor[:, 1], kind="stable")
    sel = order[N - n:]
    ids = neighbor[sel, 0].astype(np.int32)
    flags = np.ascontiguousarray(neighbor[sel, 4])
    row0 = np.ascontiguousarray(traindata[0])

    nc = _get_nc()
    in_maps = []
    for c in range(N_CORES):
        s = slice(c * E, (c + 1) * E)
        in_maps.append({
            "traindata": traindata,
            "packed": np.ascontiguousarray(_pack_core(
                ids[s].reshape(P, K), flags[s].reshape(P, K), row0)),
        })
    res = run_bass_kernel_spmd(
        nc, in_maps, core_ids=list(range(N_CORES)), trace=_trace
    )
    _cache["last_results"] = res
    out = np.concatenate([r["out"] for r in res.results], axis=0)
    return np.ascontiguousarray(out.astype(np.float32))



# revision 4
# speedup vs baseline: 1.0217x; 1.0023x over previous
"""Trainium2 Bass kernel for nn_My_maxpool1 (gnn_message_passing).

Contract: kernel(**inputs) takes FULL inputs (traindata [1.5M,4] f32,
neighbor [2M,5] f32) and returns the FULL output [262144, 4] f32,
running the gather/reduce/recurrence on 8 NeuronCores.

Host side (sharding prep, per the problem's sharding hint):
  - global stable descending sort of neighbor by col 1, keep bottom-n rows
    (this reproduces tf.nn.top_k's ordering exactly);
  - shard the n selected rows across 8 cores by contiguous group blocks;
  - each core receives the full traindata table (gather source) plus one
    packed int32 tensor [128, 2K+4]: its shard's ids (col 0 of the selected
    rows), the raw bits of their flags (col 4), and the raw bits of
    traindata[0] (replicated per partition; used for the never-updated
    sentinel groups).

Device side (per core, SPMD):
  - indirect-DMA gather rows = traindata[ids]           [128, K, 4]
  - rmax = max(rows[:, :, 1:4], axis=-1)                [128, K]
  - 4-step group recurrence (groups = consecutive fours along k):
      re = (flag_j != 0) == (rmax_j > maxmin)
      maxmin  = where(re, rmax_j, maxmin)
      maxidxf = where(re, id_j, maxidxf)
  - out rows: init traindata[0] (the clip(-100,0) sentinel case), then
    for j: where(maxidxf == id_j, rows_j) — selects traindata[maxindex]
    without a second gather.

Note: the reference's rev/td/searchsorted machinery is an exact identity
here (traindata col 0 == arange(T), all lookup keys < T, first match is
at index == key, and appended rev rows are never read back), so tp == id.
"""

import numpy as np

import concourse.bass as bass
import concourse.mybir as mybir
from concourse.bass_utils import run_bass_kernel_spmd
from concourse.tile import TileContext

F32 = mybir.dt.float32
I32 = mybir.dt.int32
U32 = mybir.dt.uint32
AX = mybir.AxisListType
OP = mybir.AluOpType

N_CORES = 8
T = 1_500_000       # traindata rows
N = 2_000_000       # neighbor rows
n = 1_048_576       # selected rows (bottom-n of the descending sort)
P = 128             # SBUF partitions
E = n // N_CORES    # selected rows per core            = 131072
K = E // P          # selected rows per partition       = 1024
NCHUNKS = 1         # single completion sem for the scalar-indirect gathers

_cache: dict = {}


def _build_nc(T=T, K=K, nchunks=NCHUNKS):
    """Raw-bass build: explicit semaphores, no TileContext.

    Tile's auto-generated kernel-tail drain accumulates one sem wait per
    DMA lane used and walrus caps sync waits per instruction at ~3, so any
    multi-DMA version of this kernel fails codegen under Tile. Raw bass
    keeps every instruction at <=1 wait.
    """
    Q = K // 4
    G_CORE = P * Q
    assert K % (4 * nchunks) == 0
    W = K // nchunks
    nc = bass.Bass()
    td = nc.declare_dram_parameter("traindata", [T, 4], F32, isOutput=False)
    # packed: [0:K) ids i32 | [K:2K) flag bits | [2K:2K+4) traindata[0] bits
    pk_d = nc.declare_dram_parameter("packed", [P, 2 * K + 4], I32,
                                     isOutput=False)
    out_d = nc.declare_dram_parameter("out", [G_CORE, 4], F32, isOutput=True)

    from contextlib import ExitStack
    with ExitStack() as ctx:
        pk = ctx.enter_context(nc.sbuf_tensor([P, 2 * K + 4], I32))
        rows_t = ctx.enter_context(nc.sbuf_tensor([P, K * 4], F32))
        rmax = ctx.enter_context(nc.sbuf_tensor([P, K], F32))
        fl = ctx.enter_context(nc.sbuf_tensor([P, K], F32))
        maxmin = ctx.enter_context(nc.sbuf_tensor([P, Q], F32))
        maxmin2 = ctx.enter_context(nc.sbuf_tensor([P, Q], F32))
        maxidxf = ctx.enter_context(nc.sbuf_tensor([P, Q], F32))
        maxidxf2 = ctx.enter_context(nc.sbuf_tensor([P, Q], F32))
        gt = ctx.enter_context(nc.sbuf_tensor([P, Q], F32))
        re = ctx.enter_context(nc.sbuf_tensor([P, Q], U32))
        mk = ctx.enter_context(nc.sbuf_tensor([P, Q], U32))
        outsb = ctx.enter_context(nc.sbuf_tensor([P, Q * 4], F32))
        in_sem = ctx.enter_context(nc.semaphore("in_sem"))
        in2_sem = ctx.enter_context(nc.semaphore("in2_sem"))
        g_sems = [ctx.enter_context(nc.semaphore(f"g_sem{i}"))
                  for i in range(16)]
        v_sem = ctx.enter_context(nc.semaphore("v_sem"))
        out_sem = ctx.enter_context(nc.semaphore("out_sem"))
        block = ctx.enter_context(nc.Block())
        ids_sb = pk[:, 0:K]
        fl_raw = pk[:, K:2 * K].bitcast(F32)
        row0 = pk[:, 2 * K:2 * K + 4].bitcast(F32)
        rows = rows_t[:].rearrange("p (k f) -> p k f", f=4)
        rows_g = rows_t[:].rearrange("p (q j f) -> p q j f", j=4, f=4)
        rmax_g = rmax[:].rearrange("p (q j) -> p q j", j=4)
        fl_g = fl[:].rearrange("p (q j) -> p q j", j=4)
        outsb_v = outsb[:].rearrange("p (q f) -> p q f", f=4)

        @block.sync
        def _(sync):
            # ids region first: the gathers gate on this half only
            sync.dma_start(out=pk[:, 0:K], in_=pk_d[:, 0:K]).then_inc(
                in_sem, 16)
            sync.dma_start(out=pk[:, K:], in_=pk_d[:, K:]).then_inc(
                in2_sem, 16)
            # one output DMA at the end: per-slice out DMAs were observed to
            # stall the gather stream (~4us per slice boundary) by contending
            # for DMA engines during the desc-gen-bound phase
            sync.wait_ge(v_sem, 16)
            sync.dma_start(
                out=out_d[:].rearrange("(p q) f -> p (q f)", p=P),
                in_=outsb[:],
            ).then_inc(out_sem, 16)
            sync.wait_ge(out_sem, 16)

        @block.gpsimd
        def _(gpsimd):
            # Scalar-dynamic-offset indirect DMA: ONE offset per partition per
            # instruction, contiguous 16B row fill. This is the only indirect
            # form this runtime executes correctly (the vector-offset path
            # returns swizzled garbage on HW); K instructions gather K rows
            # per partition.
            gpsimd.wait_ge(in_sem, 16)
            for k in range(K):
                gpsimd.indirect_dma_start(
                    out=rows_t[:, 4 * k:4 * k + 4],
                    out_offset=None,
                    in_=td[:],
                    in_offset=bass.IndirectOffsetOnAxis(
                        ap=ids_sb[:, k:k + 1], axis=0),
                ).then_inc(g_sems[k // (K // 16)], 16)

        @block.vector
        def _(vector):
            vector.wait_ge(in_sem, 16)
            vector.wait_ge(in2_sem, 16)
            vector.tensor_scalar(
                out=fl[:], in0=fl_raw, scalar1=0.0, scalar2=None,
                op0=OP.not_equal,
            )
            vector.tensor_copy(
                out=outsb_v, in_=row0.unsqueeze(1).to_broadcast([P, Q, 4]),
            )
            vector.memset(maxmin[:], -100000.0)
            vector.memset(maxidxf[:], -100.0)
            vector.drain()
            KW = K // 16         # 64 slot-cols per slice
            QW = Q // 16         # 16 group-cols per slice
            for i in range(16):
                ks = slice(KW * i, KW * (i + 1))
                qs = slice(QW * i, QW * (i + 1))
                # all of this quarter's gathers inc g_sems[i] by 16 each;
                # wait the full quarter count (no partial waits: completion
                # order across instructions is not guaranteed)
                vector.wait_ge(g_sems[i], 16 * (K // 16))
                vector.tensor_reduce(
                    rmax[:, ks], rows[:, ks, 1:4], AX.X, OP.max
                )
                vector.drain()
                cur_mm, nxt_mm = maxmin, maxmin2
                cur_mi, nxt_mi = maxidxf, maxidxf2
                for j in range(4):
                    vector.tensor_tensor(
                        out=gt[:, qs], in0=rmax_g[:, qs, j],
                        in1=cur_mm[:, qs], op=OP.is_gt
                    )
                    vector.drain()
                    vector.tensor_tensor(
                        out=re[:, qs], in0=fl_g[:, qs, j], in1=gt[:, qs],
                        op=OP.is_equal
                    )
                    vector.drain()
                    vector.select(
                        out=nxt_mm[:, qs], mask=re[:, qs],
                        on_true=rmax_g[:, qs, j], on_false=cur_mm[:, qs],
                        add_drain=True,
                    )
                    vector.select(
                        out=nxt_mi[:, qs], mask=re[:, qs],
                        on_true=rows_g[:, qs, j, 0], on_false=cur_mi[:, qs],
                        add_drain=True,
                    )
                    vector.drain()
                    cur_mm, nxt_mm = nxt_mm, cur_mm
                    cur_mi, nxt_mi = nxt_mi, cur_mi
                for j in range(4):
                    vector.tensor_tensor(
                        out=mk[:, qs], in0=cur_mi[:, qs],
                        in1=rows_g[:, qs, j, 0], op=OP.is_equal
                    )
                    vector.drain()
                    for f in range(4):
                        vector.copy_predicated(
                            out=outsb_v[:, qs, f],
                            mask=mk[:, qs],
                            data=rows_g[:, qs, j, f],
                        )
                    vector.drain()
                vector.nop().then_inc(v_sem, 1)

    return nc


def _build_nc_tile(T=T, K=K, nchunks=NCHUNKS):
    Q = K // 4
    G_CORE = P * Q
    assert K % (4 * nchunks) == 0
    W = K // nchunks
    nc = bass.Bass()
    td = nc.declare_dram_parameter("traindata", [T, 4], F32, isOutput=False)
    # packed: [0:K) ids i32 | [K:2K) flag bits | [2K:2K+4) traindata[0] bits
    pk_d = nc.declare_dram_parameter("packed", [P, 2 * K + 4], I32,
                                     isOutput=False)
    out_d = nc.declare_dram_parameter("out", [G_CORE, 4], F32, isOutput=True)

    with TileContext(nc) as tc:
        with tc.tile_pool(name="main", bufs=1) as pool:
            pk = pool.tile([P, 2 * K + 4], I32)
            nc.sync.dma_start(out=pk[:], in_=pk_d[:])
            ids_sb = pk[:, 0:K]
            fl_raw = pk[:, K:2 * K].bitcast(F32)
            row0 = pk[:, 2 * K:2 * K + 4].bitcast(F32)

            # main gather, chunked so SWDGE descriptor gen overlaps transfers
            rows = pool.tile([P, K, 4], F32)
            for i in range(nchunks):
                rs = slice(i * W, (i + 1) * W)
                nc.gpsimd.indirect_dma_start(
                    out=rows[:, rs, :],
                    out_offset=None,
                    in_=td[:],
                    in_offset=bass.IndirectOffsetOnAxis(ap=ids_sb[:, rs], axis=0),
                )

            # rmax over feature cols 1:4, per chunk (overlaps later gathers)
            rmax = pool.tile([P, K], F32)
            for i in range(nchunks):
                rs = slice(i * W, (i + 1) * W)
                nc.vector.tensor_reduce(
                    rmax[:, rs], rows[:, rs, 1:4], AX.X, OP.max
                )

            fl = pool.tile([P, K], F32)
            nc.vector.tensor_scalar(
                out=fl[:], in0=fl_raw, scalar1=0.0, scalar2=None,
                op0=OP.not_equal,
            )

            # group views: [P, Q, 4] over k = 4q + j
            rmax_g = rmax[:].rearrange("p (q j) -> p q j", j=4)
            fl_g = fl[:].rearrange("p (q j) -> p q j", j=4)
            rows_g = rows[:].rearrange("p (q j) f -> p q j f", j=4)

            maxmin = pool.tile([P, Q], F32)
            nc.vector.memset(maxmin[:], -100000.0)
            maxidxf = pool.tile([P, Q], F32)
            nc.vector.memset(maxidxf[:], -100.0)
            gt = pool.tile([P, Q], F32)
            re = pool.tile([P, Q], U32)
            maxmin2 = pool.tile([P, Q], F32)
            maxidxf2 = pool.tile([P, Q], F32)

            cur_mm, nxt_mm = maxmin, maxmin2
            cur_mi, nxt_mi = maxidxf, maxidxf2
            for j in range(4):
                nc.vector.tensor_tensor(
                    out=gt[:], in0=rmax_g[:, :, j], in1=cur_mm[:], op=OP.is_gt
                )
                nc.vector.tensor_tensor(
                    out=re[:], in0=fl_g[:, :, j], in1=gt[:], op=OP.is_equal
                )
                nc.vector.select(
                    out=nxt_mm[:], mask=re[:],
                    on_true=rmax_g[:, :, j], on_false=cur_mm[:],
                )
                nc.vector.select(
                    out=nxt_mi[:], mask=re[:],
                    on_true=rows_g[:, :, j, 0], on_false=cur_mi[:],
                )
                cur_mm, nxt_mm = nxt_mm, cur_mm
                cur_mi, nxt_mi = nxt_mi, cur_mi

            # output rows
            outsb = pool.tile([P, Q * 4], F32)
            outsb_v = outsb[:].rearrange("p (q f) -> p q f", f=4)
            nc.vector.tensor_copy(
                out=outsb_v,
                in_=row0.unsqueeze(1).to_broadcast([P, Q, 4]),
            )
            mk = pool.tile([P, Q], U32)
            for j in range(4):
                nc.vector.tensor_tensor(
                    out=mk[:], in0=cur_mi[:], in1=rows_g[:, :, j, 0], op=OP.is_equal
                )
                for f in range(4):
                    nc.vector.copy_predicated(
                        out=outsb_v[:, :, f],
                        mask=mk[:],
                        data=rows_g[:, :, j, f],
                    )

            nc.sync.dma_start(
                out=out_d[:].rearrange("(p q) f -> p (q f)", p=P),
                in_=outsb[:],
            )

    return nc


def _get_nc():
    if "nc" not in _cache:
        _cache["nc"] = _build_nc()
    return _cache["nc"]


def _pack_core(ids_i32, flags_f32, row0_f32):
    """[P, K] int32 ids, [P, K] f32 flags, [4] f32 row0 -> [P, 2K+4] int32."""
    return np.concatenate(
        [ids_i32,
         flags_f32.view(np.int32),
         np.broadcast_to(row0_f32.view(np.int32), (P, 4))],
        axis=1,
    )


def kernel(traindata, neighbor, _trace=False):
    traindata = np.ascontiguousarray(np.asarray(traindata, dtype=np.float32))
    neighbor = np.asarray(neighbor, dtype=np.float32)
    assert traindata.shape == (T, 4) and neighbor.shape == (N, 5)

    # ---- host: global sort + shard (the sharding hint's "after the global
    # sort" prep) ----
    order = np.argsort(-neighbor[:, 1], kind="stable")
    sel = order[N - n:]
    ids = neighbor[sel, 0].astype(np.int32)
    flags = np.ascontiguousarray(neighbor[sel, 4])
    row0 = np.ascontiguousarray(traindata[0])

    nc = _get_nc()
    in_maps = []
    for c in range(N_CORES):
        s = slice(c * E, (c + 1) * E)
        in_maps.append({
            "traindata": traindata,
            "packed": np.ascontiguousarray(_pack_core(
                ids[s].reshape(P, K), flags[s].reshape(P, K), row0)),
        })
    res = run_bass_kernel_spmd(
        nc, in_maps, core_ids=list(range(N_CORES)), trace=_trace
    )
    _cache["last_results"] = res
    out = np.concatenate([r["out"] for r in res.results], axis=0)
    return np.ascontiguousarray(out.astype(np.float32))



# revision 5
# speedup vs baseline: 1.0238x; 1.0021x over previous
"""Trainium2 Bass kernel for nn_My_maxpool1 (gnn_message_passing).

Contract: kernel(**inputs) takes FULL inputs (traindata [1.5M,4] f32,
neighbor [2M,5] f32) and returns the FULL output [262144, 4] f32,
running the gather/reduce/recurrence on 8 NeuronCores.

Host side (sharding prep, per the problem's sharding hint):
  - global stable descending sort of neighbor by col 1, keep bottom-n rows
    (this reproduces tf.nn.top_k's ordering exactly);
  - shard the n selected rows across 8 cores by contiguous group blocks;
  - each core receives the full traindata table (gather source) plus one
    packed int32 tensor [128, 2K+4]: its shard's ids (col 0 of the selected
    rows), the raw bits of their flags (col 4), and the raw bits of
    traindata[0] (replicated per partition; used for the never-updated
    sentinel groups).

Device side (per core, SPMD):
  - indirect-DMA gather rows = traindata[ids]           [128, K, 4]
  - rmax = max(rows[:, :, 1:4], axis=-1)                [128, K]
  - 4-step group recurrence (groups = consecutive fours along k):
      re = (flag_j != 0) == (rmax_j > maxmin)
      maxmin  = where(re, rmax_j, maxmin)
      maxidxf = where(re, id_j, maxidxf)
  - out rows: init traindata[0] (the clip(-100,0) sentinel case), then
    for j: where(maxidxf == id_j, rows_j) — selects traindata[maxindex]
    without a second gather.

Note: the reference's rev/td/searchsorted machinery is an exact identity
here (traindata col 0 == arange(T), all lookup keys < T, first match is
at index == key, and appended rev rows are never read back), so tp == id.
"""

import numpy as np

import concourse.bass as bass
import concourse.mybir as mybir
from concourse.bass_utils import run_bass_kernel_spmd
from concourse.tile import TileContext

F32 = mybir.dt.float32
I32 = mybir.dt.int32
U32 = mybir.dt.uint32
AX = mybir.AxisListType
OP = mybir.AluOpType

N_CORES = 8
T = 1_500_000       # traindata rows
N = 2_000_000       # neighbor rows
n = 1_048_576       # selected rows (bottom-n of the descending sort)
P = 128             # SBUF partitions
E = n // N_CORES    # selected rows per core            = 131072
K = E // P          # selected rows per partition       = 1024
NCHUNKS = 1         # single completion sem for the scalar-indirect gathers

_cache: dict = {}


def _build_nc(T=T, K=K, nchunks=NCHUNKS):
    """Raw-bass build: explicit semaphores, no TileContext.

    Tile's auto-generated kernel-tail drain accumulates one sem wait per
    DMA lane used and walrus caps sync waits per instruction at ~3, so any
    multi-DMA version of this kernel fails codegen under Tile. Raw bass
    keeps every instruction at <=1 wait.
    """
    Q = K // 4
    G_CORE = P * Q
    assert K % (4 * nchunks) == 0
    W = K // nchunks
    nc = bass.Bass()
    td = nc.declare_dram_parameter("traindata", [T, 4], F32, isOutput=False)
    # packed: [0:K) ids i32 | [K:2K) flag bits | [2K:2K+4) traindata[0] bits
    pk_d = nc.declare_dram_parameter("packed", [P, 2 * K + 4], I32,
                                     isOutput=False)
    out_d = nc.declare_dram_parameter("out", [G_CORE, 4], F32, isOutput=True)

    from contextlib import ExitStack
    with ExitStack() as ctx:
        pk = ctx.enter_context(nc.sbuf_tensor([P, 2 * K + 4], I32))
        rows_t = ctx.enter_context(nc.sbuf_tensor([P, K * 4], F32))
        rmax = ctx.enter_context(nc.sbuf_tensor([P, K], F32))
        fl = ctx.enter_context(nc.sbuf_tensor([P, K], F32))
        maxmin = ctx.enter_context(nc.sbuf_tensor([P, Q], F32))
        maxmin2 = ctx.enter_context(nc.sbuf_tensor([P, Q], F32))
        maxidxf = ctx.enter_context(nc.sbuf_tensor([P, Q], F32))
        maxidxf2 = ctx.enter_context(nc.sbuf_tensor([P, Q], F32))
        gt = ctx.enter_context(nc.sbuf_tensor([P, Q], F32))
        re = ctx.enter_context(nc.sbuf_tensor([P, Q], U32))
        mk = ctx.enter_context(nc.sbuf_tensor([P, Q], U32))
        outsb = ctx.enter_context(nc.sbuf_tensor([P, Q * 4], F32))
        in_sem = ctx.enter_context(nc.semaphore("in_sem"))
        in2_sem = ctx.enter_context(nc.semaphore("in2_sem"))
        g_sems = [ctx.enter_context(nc.semaphore(f"g_sem{i}"))
                  for i in range(16)]
        v_sem = ctx.enter_context(nc.semaphore("v_sem"))
        g15b_sem = ctx.enter_context(nc.semaphore("g15b_sem"))
        out_sem = ctx.enter_context(nc.semaphore("out_sem"))
        block = ctx.enter_context(nc.Block())
        ids_sb = pk[:, 0:K]
        fl_raw = pk[:, K:2 * K].bitcast(F32)
        row0 = pk[:, 2 * K:2 * K + 4].bitcast(F32)
        rows = rows_t[:].rearrange("p (k f) -> p k f", f=4)
        rows_g = rows_t[:].rearrange("p (q j f) -> p q j f", j=4, f=4)
        rmax_g = rmax[:].rearrange("p (q j) -> p q j", j=4)
        fl_g = fl[:].rearrange("p (q j) -> p q j", j=4)
        outsb_v = outsb[:].rearrange("p (q f) -> p q f", f=4)

        @block.sync
        def _(sync):
            # ids region first: the gathers gate on this half only
            sync.dma_start(out=pk[:, 0:K], in_=pk_d[:, 0:K]).then_inc(
                in_sem, 16)
            sync.dma_start(out=pk[:, K:], in_=pk_d[:, K:]).then_inc(
                in2_sem, 16)
            # one output DMA at the end: per-slice out DMAs were observed to
            # stall the gather stream (~4us per slice boundary) by contending
            # for DMA engines during the desc-gen-bound phase
            sync.wait_ge(v_sem, 16)
            sync.dma_start(
                out=out_d[:].rearrange("(p q) f -> p (q f)", p=P),
                in_=outsb[:],
            ).then_inc(out_sem, 16)
            sync.wait_ge(out_sem, 16)

        @block.gpsimd
        def _(gpsimd):
            # Scalar-dynamic-offset indirect DMA: ONE offset per partition per
            # instruction, contiguous 16B row fill. This is the only indirect
            # form this runtime executes correctly (the vector-offset path
            # returns swizzled garbage on HW); K instructions gather K rows
            # per partition.
            gpsimd.wait_ge(in_sem, 16)
            # last slice issued j-interleaved: members j=0,1 of its groups
            # first (phase A -> g_sems[15]), then j=2,3 (phase B -> g15b),
            # so the final slice's reduce/recurrence pipelines under phase B.
            # Destinations are unchanged (instr k always writes column k).
            SL = K - K // 16
            order = (list(range(SL))
                     + [SL + c for c in range(K // 16) if c % 4 < 2]
                     + [SL + c for c in range(K // 16) if c % 4 >= 2])
            for n, k in enumerate(order):
                if k < SL:
                    sem = g_sems[k // (K // 16)]
                elif n < SL + K // 32:
                    sem = g_sems[15]
                else:
                    sem = g15b_sem
                gpsimd.indirect_dma_start(
                    out=rows_t[:, 4 * k:4 * k + 4],
                    out_offset=None,
                    in_=td[:],
                    in_offset=bass.IndirectOffsetOnAxis(
                        ap=ids_sb[:, k:k + 1], axis=0),
                ).then_inc(sem, 16)

        @block.vector
        def _(vector):
            vector.wait_ge(in_sem, 16)
            vector.wait_ge(in2_sem, 16)
            vector.tensor_scalar(
                out=fl[:], in0=fl_raw, scalar1=0.0, scalar2=None,
                op0=OP.not_equal,
            )
            vector.tensor_copy(
                out=outsb_v, in_=row0.unsqueeze(1).to_broadcast([P, Q, 4]),
            )
            vector.memset(maxmin[:], -100000.0)
            vector.memset(maxidxf[:], -100.0)
            vector.drain()
            KW = K // 16         # 64 slot-cols per slice
            QW = Q // 16         # 16 group-cols per slice
            for i in range(16):
                ks = slice(KW * i, KW * (i + 1))
                qs = slice(QW * i, QW * (i + 1))
                # all of this quarter's gathers inc g_sems[i] by 16 each;
                # wait the full quarter count (no partial waits: completion
                # order across instructions is not guaranteed)
                if i < 15:
                    vector.wait_ge(g_sems[i], 16 * (K // 16))
                    vector.tensor_reduce(
                        rmax[:, ks], rows[:, ks, 1:4], AX.X, OP.max
                    )
                else:
                    vector.wait_ge(g_sems[i], 16 * (K // 32))
                    vector.tensor_reduce(
                        rmax_g[:, qs, 0:2],
                        rows_g[:, qs, 0:2, 1:4], AX.X, OP.max
                    )
                vector.drain()
                cur_mm, nxt_mm = maxmin, maxmin2
                cur_mi, nxt_mi = maxidxf, maxidxf2
                for j in range(4):
                    if i == 15 and j == 2:
                        vector.wait_ge(g15b_sem, 16 * (K // 32))
                        vector.tensor_reduce(
                            rmax_g[:, qs, 2:4],
                            rows_g[:, qs, 2:4, 1:4], AX.X, OP.max
                        )
                        vector.drain()
                    vector.tensor_tensor(
                        out=gt[:, qs], in0=rmax_g[:, qs, j],
                        in1=cur_mm[:, qs], op=OP.is_gt
                    )
                    vector.drain()
                    vector.tensor_tensor(
                        out=re[:, qs], in0=fl_g[:, qs, j], in1=gt[:, qs],
                        op=OP.is_equal
                    )
                    vector.drain()
                    vector.select(
                        out=nxt_mm[:, qs], mask=re[:, qs],
                        on_true=rmax_g[:, qs, j], on_false=cur_mm[:, qs],
                        add_drain=True,
                    )
                    vector.select(
                        out=nxt_mi[:, qs], mask=re[:, qs],
                        on_true=rows_g[:, qs, j, 0], on_false=cur_mi[:, qs],
                        add_drain=True,
                    )
                    vector.drain()
                    cur_mm, nxt_mm = nxt_mm, cur_mm
                    cur_mi, nxt_mi = nxt_mi, cur_mi
                for j in range(4):
                    vector.tensor_tensor(
                        out=mk[:, qs], in0=cur_mi[:, qs],
                        in1=rows_g[:, qs, j, 0], op=OP.is_equal
                    )
                    vector.drain()
                    for f in range(4):
                        vector.copy_predicated(
                            out=outsb_v[:, qs, f],
                            mask=mk[:, qs],
                            data=rows_g[:, qs, j, f],
                        )
                    vector.drain()
                vector.nop().then_inc(v_sem, 1)

    return nc


def _build_nc_tile(T=T, K=K, nchunks=NCHUNKS):
    Q = K // 4
    G_CORE = P * Q
    assert K % (4 * nchunks) == 0
    W = K // nchunks
    nc = bass.Bass()
    td = nc.declare_dram_parameter("traindata", [T, 4], F32, isOutput=False)
    # packed: [0:K) ids i32 | [K:2K) flag bits | [2K:2K+4) traindata[0] bits
    pk_d = nc.declare_dram_parameter("packed", [P, 2 * K + 4], I32,
                                     isOutput=False)
    out_d = nc.declare_dram_parameter("out", [G_CORE, 4], F32, isOutput=True)

    with TileContext(nc) as tc:
        with tc.tile_pool(name="main", bufs=1) as pool:
            pk = pool.tile([P, 2 * K + 4], I32)
            nc.sync.dma_start(out=pk[:], in_=pk_d[:])
            ids_sb = pk[:, 0:K]
            fl_raw = pk[:, K:2 * K].bitcast(F32)
            row0 = pk[:, 2 * K:2 * K + 4].bitcast(F32)

            # main gather, chunked so SWDGE descriptor gen overlaps transfers
            rows = pool.tile([P, K, 4], F32)
            for i in range(nchunks):
                rs = slice(i * W, (i + 1) * W)
                nc.gpsimd.indirect_dma_start(
                    out=rows[:, rs, :],
                    out_offset=None,
                    in_=td[:],
                    in_offset=bass.IndirectOffsetOnAxis(ap=ids_sb[:, rs], axis=0),
                )

            # rmax over feature cols 1:4, per chunk (overlaps later gathers)
            rmax = pool.tile([P, K], F32)
            for i in range(nchunks):
                rs = slice(i * W, (i + 1) * W)
                nc.vector.tensor_reduce(
                    rmax[:, rs], rows[:, rs, 1:4], AX.X, OP.max
                )

            fl = pool.tile([P, K], F32)
            nc.vector.tensor_scalar(
                out=fl[:], in0=fl_raw, scalar1=0.0, scalar2=None,
                op0=OP.not_equal,
            )

            # group views: [P, Q, 4] over k = 4q + j
            rmax_g = rmax[:].rearrange("p (q j) -> p q j", j=4)
            fl_g = fl[:].rearrange("p (q j) -> p q j", j=4)
            rows_g = rows[:].rearrange("p (q j) f -> p q j f", j=4)

            maxmin = pool.tile([P, Q], F32)
            nc.vector.memset(maxmin[:], -100000.0)
            maxidxf = pool.tile([P, Q], F32)
            nc.vector.memset(maxidxf[:], -100.0)
            gt = pool.tile([P, Q], F32)
            re = pool.tile([P, Q], U32)
            maxmin2 = pool.tile([P, Q], F32)
            maxidxf2 = pool.tile([P, Q], F32)

            cur_mm, nxt_mm = maxmin, maxmin2
            cur_mi, nxt_mi = maxidxf, maxidxf2
            for j in range(4):
                nc.vector.tensor_tensor(
                    out=gt[:], in0=rmax_g[:, :, j], in1=cur_mm[:], op=OP.is_gt
                )
                nc.vector.tensor_tensor(
                    out=re[:], in0=fl_g[:, :, j], in1=gt[:], op=OP.is_equal
                )
                nc.vector.select(
                    out=nxt_mm[:], mask=re[:],
                    on_true=rmax_g[:, :, j], on_false=cur_mm[:],
                )
                nc.vector.select(
                    out=nxt_mi[:], mask=re[:],
                    on_true=rows_g[:, :, j, 0], on_false=cur_mi[:],
                )
                cur_mm, nxt_mm = nxt_mm, cur_mm
                cur_mi, nxt_mi = nxt_mi, cur_mi

            # output rows
            outsb = pool.tile([P, Q * 4], F32)
            outsb_v = outsb[:].rearrange("p (q f) -> p q f", f=4)
            nc.vector.tensor_copy(
                out=outsb_v,
                in_=row0.unsqueeze(1).to_broadcast([P, Q, 4]),
            )
            mk = pool.tile([P, Q], U32)
            for j in range(4):
                nc.vector.tensor_tensor(
                    out=mk[:], in0=cur_mi[:], in1=rows_g[:, :, j, 0], op=OP.is_equal
                )
                for f in range(4):
                    nc.vector.copy_predicated(
                        out=outsb_v[:, :, f],
                        mask=mk[:],
                        data=rows_g[:, :, j, f],
                    )

            nc.sync.dma_start(
                out=out_d[:].rearrange("(p q) f -> p (q f)", p=P),
                in_=outsb[:],
            )

    return nc


def _get_nc():
    if "nc" not in _cache:
        _cache["nc"] = _build_nc()
    return _cache["nc"]


def _pack_core(ids_i32, flags_f32, row0_f32):
    """[P, K] int32 ids, [P, K] f32 flags, [4] f32 row0 -> [P, 2K+4] int32."""
    return np.concatenate(
        [ids_i32,
         flags_f32.view(np.int32),
         np.broadcast_to(row0_f32.view(np.int32), (P, 4))],
        axis=1,
    )


def kernel(traindata, neighbor, _trace=False):
    traindata = np.ascontiguousarray(np.asarray(traindata, dtype=np.float32))
    neighbor = np.asarray(neighbor, dtype=np.float32)
    assert traindata.shape == (T, 4) and neighbor.shape == (N, 5)

    # ---- host: global sort + shard (the sharding hint's "after the global
    # sort" prep) ----
    order = np.argsort(-neighbor[:, 1], kind="stable")
    sel = order[N - n:]
    ids = neighbor[sel, 0].astype(np.int32)
    flags = np.ascontiguousarray(neighbor[sel, 4])
    row0 = np.ascontiguousarray(traindata[0])

    nc = _get_nc()
    in_maps = []
    for c in range(N_CORES):
        s = slice(c * E, (c + 1) * E)
        in_maps.append({
            "traindata": traindata,
            "packed": np.ascontiguousarray(_pack_core(
                ids[s].reshape(P, K), flags[s].reshape(P, K), row0)),
        })
    res = run_bass_kernel_spmd(
        nc, in_maps, core_ids=list(range(N_CORES)), trace=_trace
    )
    _cache["last_results"] = res
    out = np.concatenate([r["out"] for r in res.results], axis=0)
    return np.ascontiguousarray(out.astype(np.float32))

